# revision 42
# baseline (speedup 1.0000x reference)
"""GTLayer (gnn_message_passing) Trainium2 kernel, v7.

Core-uniform single program, dest-sharded across 8 cores (12500 dest
nodes / ~75k edges per core; no collectives needed).

Per core: ~100 windows (<=128 dest segments x <=768 edges, 128-node window).

Host prep (layout only - gathers/permutes/transposes of input rows,
no arithmetic on values):
  - ctabT [G, 128, 2, 768] fp16: per-window TRANSPOSED per-edge-slot
    embeddings: [:, :, 0, e] = embeds[col_e], [:, :, 1, e] = embeds[dest_e].
  - ohE [G, 128, 6, 132] bf16: one-hot scatter matrix (cols 0:128) and
    filt[col_e] (cols 128:132), both in edge-partition layout.
All streamed to SBUF with plain HWDGE DMAs - no GPSIMD, no device-side
gathers (measured: SWDGE descriptor generation costs ~8ns/row on HW,
capping any device-gather design at ~650us for 75k edges/core).

Device per window (edge slots on the free axis, transposed layout):
  - qeT = qW^T @ destT, keT = kW^T @ colT (PSUM f32, 2 matmuls each)
  - ACT evacuates qeT -> SBUF fp16 (TT reads at most one PSUM input)
  - qkT = keT * qeT (DVE, fp16)
  - att[e, h] directly per 128-edge tile: matmul(lhsT=qkT chunk,
    rhs=hsel head-selector) -> acc[:, 132:156] - no [h, e] intermediate,
    no transpose-back matmuls
  - DVE clip(+-10) + filt add ([128, 24]); ACT exp -> rhs[:, :, 128:132]
  - ve = colT^T @ vW (PSUM, 6 matmuls); ACT evacuates ve -> SBUF bf16
    (rhs is off the critical path - the scatter delay absorbs the hop);
    DVE rhs = ve * expatt (bf16, all-SBUF)
  - scatter acc[s, 0:132] += oh^T @ [rhs | expatt] (6 matmuls, PSUM f32
    accumulation), delayed one window so the PE stays fed while DVE
    finishes the current window's rhs/oh
  - normalize by acc[:, 128:132] + 1e-8; fp16 out DMA (host casts back).
Host unpermutes window/segment rows to node rows at the end.

History: v2 (953us) per-edge indirect gathers + PE transposes; v4 (742us)
transposed dma_gather, GPSIMD-bound; v5 (515us) host-pretransposed
streams; v6 (313us) direct att matmuls; v7 (300us) merged oh/filt
stream + fp16 output; v12 (292us) split qts/qkT half-tiles + ACT ve
evacuation for an all-SBUF rhs multiply. Measured dead ends: Pool
engine rejects/corrupts elementwise ops; ACT hops on the latency-
critical qkT path regress; merging streams or PSUM tiles coarsens
dependencies and regresses; fp8 blocked by exp range / logit precision.
"""

import numpy as np
import ml_dtypes

N = 100000
E = 600000
LATDIM = 128
HEAD = 4
HDIM = 32
NCORES = 8
NLOC = N // NCORES              # 12500
CAP_S = 128
K_TILES = 6
CAP_E = K_TILES * 128           # 768
PAD_SEG = 999.0

f16 = np.float16
bf16np = ml_dtypes.bfloat16

_CACHE = {}


# --------------------------------------------------------------------------
# host-side planning
# --------------------------------------------------------------------------

def _plan_core(rows, cols, base):
    sel = (rows >= base) & (rows < base + NLOC)
    r = rows[sel].astype(np.int64) - base
    c = cols[sel].astype(np.int64)
    o = np.argsort(r, kind="stable")
    r, c = r[o], c[o]
    seg_nodes, seg_starts, seg_counts = np.unique(
        r, return_index=True, return_counts=True
    )
    nseg = len(seg_nodes)
    bounds = []
    lo, cur = 0, 0
    for i in range(nseg):
        cnt = int(seg_counts[i])
        if ((i - lo) + 1 > CAP_S or cur + cnt > CAP_E
                or seg_nodes[i] - seg_nodes[lo] >= CAP_S):
            bounds.append((lo, i))
            lo, cur = i, 0
        cur += cnt
    bounds.append((lo, nseg))
    ngroups = len(bounds)

    cidx = np.zeros((ngroups, CAP_E), dtype=np.int64)
    didx = np.zeros((ngroups, CAP_E), dtype=np.int64)   # dest node (global)
    segrel = np.full((ngroups, CAP_E), PAD_SEG, dtype=np.float32)
    remap_rows, remap_nodes = [], []
    for g, (slo, shi) in enumerate(bounds):
        e_lo = int(seg_starts[slo])
        e_hi = int(seg_starts[shi]) if shi < nseg else len(r)
        ne = e_hi - e_lo
        cidx[g, :ne] = c[e_lo:e_hi]
        didx[g, :ne] = r[e_lo:e_hi] + base
        rel = r[e_lo:e_hi] - seg_nodes[slo]
        segrel[g, :ne] = rel.astype(np.float32)
        remap_rows.append(g * CAP_S + (seg_nodes[slo:shi] - seg_nodes[slo]))
        remap_nodes.append(seg_nodes[slo:shi])
    return dict(
        ngroups=ngroups, cidx=cidx, didx=didx, segrel=segrel,
        remap_rows=np.concatenate(remap_rows),
        remap_nodes=np.concatenate(remap_nodes),
    )


# --------------------------------------------------------------------------
# device program
# --------------------------------------------------------------------------

def _build_nc(G):
    import concourse.bass as bass
    import concourse.mybir as mybir
    import concourse.tile as tile
    from concourse import bacc

    fp32 = mybir.dt.float32
    fp16 = mybir.dt.float16
    bf16 = mybir.dt.bfloat16

    nc = bacc.Bacc(None, target_bir_lowering=False, debug=True)

    ctabT = nc.dram_tensor("ctabT", [G, 128, 2, CAP_E], fp16,
                           kind="ExternalInput")
    ohE = nc.dram_tensor("ohE", [G, 128, K_TILES, 132], bf16,
                         kind="ExternalInput")
    qW = nc.dram_tensor("qW", [LATDIM, LATDIM], fp16, kind="ExternalInput")
    kW = nc.dram_tensor("kW", [LATDIM, LATDIM], fp16, kind="ExternalInput")
    vW = nc.dram_tensor("vW", [LATDIM, LATDIM], fp16, kind="ExternalInput")
    hsel = nc.dram_tensor("hsel", [LATDIM, HEAD], fp16, kind="ExternalInput")
    res = nc.dram_tensor("res", [G * CAP_S, LATDIM], fp16,
                         kind="ExternalOutput")

    with tile.TileContext(nc) as tc:
        with (
            tc.tile_pool(name="const", bufs=1) as constp,
            tc.tile_pool(name="gat", bufs=6) as gatp,
            tc.tile_pool(name="ohp", bufs=6) as ohp,
            tc.tile_pool(name="work", bufs=6) as workp,
            tc.tile_pool(name="ae", bufs=6) as aep,
            tc.tile_pool(name="wb", bufs=6) as wbp,
            tc.tile_pool(name="outp", bufs=6) as outp,
            tc.tile_pool(name="kv", bufs=3, space="PSUM") as kvp,
            tc.tile_pool(name="accps", bufs=2, space="PSUM") as accp,
        ):
            # ---- constants ----
            qW_sb = constp.tile([128, 128], fp16, tag="qW")
            nc.sync.dma_start(qW_sb[:], qW[:])
            kW_sb = constp.tile([128, 128], fp16, tag="kW")
            nc.sync.dma_start(kW_sb[:], kW[:])
            vW_sb = constp.tile([128, 128], fp16, tag="vW")
            nc.sync.dma_start(vW_sb[:], vW[:])
            hsel_sb = constp.tile([128, HEAD], fp16, tag="hsel")
            nc.sync.dma_start(hsel_sb[:], hsel[:])

            pend = None
            for g in range(G):
                ceT = gatp.tile([128, 2, CAP_E], fp16, tag="ceT")
                nc.sync.dma_start(ceT[:], ctabT[g])
                ohf = ohp.tile([128, K_TILES, 132], bf16, tag="ohf")
                nc.sync.dma_start(ohf[:], ohE[g])
                oh = ohf[:, :, 0:128]
                fe = ohf[:, :, 128:132]

                # pending scatter first: its inputs completed a window ago,
                # so the PE has ready work while this window's stream lands
                if pend is not None:
                    _emit_scatter(nc, mybir, outp, res, *pend)

                # qeT / keT (PSUM f32)
                qt_a = kvp.tile([128, 512], fp32, tag="KA")
                qt_b = kvp.tile([128, 256], fp32, tag="KB")
                nc.tensor.matmul(qt_a[:], qW_sb[:], ceT[:, 1, 0:512],
                                 start=True, stop=True)
                nc.tensor.matmul(qt_b[:], qW_sb[:], ceT[:, 1, 512:768],
                                 start=True, stop=True)
                kt_a = kvp.tile([128, 512], fp32, tag="KA")
                kt_b = kvp.tile([128, 256], fp32, tag="KB")
                nc.tensor.matmul(kt_a[:], kW_sb[:], ceT[:, 0, 0:512],
                                 start=True, stop=True)
                nc.tensor.matmul(kt_b[:], kW_sb[:], ceT[:, 0, 512:768],
                                 start=True, stop=True)
                # evac qt to SBUF (ACT) — TT may read only one PSUM input;
                # split tiles so att tiles 0-3 depend only on the a-half
                qts_a = workp.tile([128, 512], fp16, tag="qtsa")
                qts_b = workp.tile([128, 256], fp16, tag="qtsb")
                nc.scalar.copy(qts_a[:], qt_a[:])
                nc.scalar.copy(qts_b[:], qt_b[:])
                # qkT (DVE, fp16)
                qkT_a = workp.tile([128, 512], fp16, tag="qkta")
                qkT_b = workp.tile([128, 256], fp16, tag="qktb")
                nc.vector.tensor_tensor(qkT_a[:], kt_a[:], qts_a[:],
                                        op=mybir.AluOpType.mult)
                nc.vector.tensor_tensor(qkT_b[:], kt_b[:], qts_b[:],
                                        op=mybir.AluOpType.mult)
                # ve (PSUM)
                ve_a = kvp.tile([128, 512], fp32, tag="KA")
                ve_b = kvp.tile([128, 256], fp32, tag="KB")
                for t in range(K_TILES):
                    if t < 4:
                        vout = ve_a[:, t * 128:(t + 1) * 128]
                    else:
                        vout = ve_b[:, (t - 4) * 128:(t - 3) * 128]
                    nc.tensor.matmul(
                        vout, ceT[:, 0, t * 128:(t + 1) * 128], vW_sb[:],
                        start=True, stop=True,
                    )
                # att[e, h] directly: per tile, lhsT=qkT chunk, rhs=hsel
                acc_ps = accp.tile([128, 156], fp32, tag="acc")
                for t in range(K_TILES):
                    if t < 4:
                        lhs = qkT_a[:, t * 128:(t + 1) * 128]
                    else:
                        lhs = qkT_b[:, (t - 4) * 128:(t - 3) * 128]
                    nc.tensor.matmul(
                        acc_ps[:, 132 + t * 4:132 + (t + 1) * 4],
                        lhs,
                        hsel_sb[:],
                        start=True, stop=True,
                    )
                # clip + filt (DVE, [128, 24])
                ae = aep.tile([128, K_TILES, HEAD], fp16, tag="ae")
                nc.vector.tensor_scalar(
                    ae[:],
                    acc_ps[:, 132:156].rearrange("p (t h) -> p t h", h=HEAD),
                    10.0, -10.0,
                    op0=mybir.AluOpType.min, op1=mybir.AluOpType.max,
                )
                nc.vector.tensor_tensor(ae[:], ae[:], fe,
                                        op=mybir.AluOpType.add)
                rhs = wbp.tile([128, K_TILES, LATDIM + HEAD], bf16, tag="rhs")
                nc.scalar.activation(
                    rhs[:, :, 128:132], ae[:],
                    mybir.ActivationFunctionType.Exp,
                )

                # evac ve to SBUF bf16 (ACT) so the rhs mult gets the DVE
                # all-SBUF fast mode; the one-window scatter delay absorbs
                # the extra latency hop
                ves = wbp.tile([128, CAP_E], bf16, tag="ves")
                nc.scalar.copy(ves[:, 0:512], ve_a[:])
                nc.scalar.copy(ves[:, 512:768], ve_b[:])
                # rhs = ve * expatt (DVE, all-SBUF)
                nc.vector.tensor_tensor(
                    rhs[:, :, 0:128].rearrange("p t (h d) -> p t h d", h=HEAD),
                    ves[:].rearrange("p (t h d) -> p t h d", t=K_TILES, h=HEAD),
                    rhs[:, :, 128:132].rearrange("p t (h o) -> p t h o", o=1)
                    .to_broadcast([128, K_TILES, HEAD, HDIM]),
                    op=mybir.AluOpType.mult,
                )
                pend = (g, acc_ps, oh, rhs)
            _emit_scatter(nc, mybir, outp, res, *pend)

    nc.compile()
    return nc


def _emit_scatter(nc, mybir, outp, res, g, acc_ps, oh, rhs):
    for t in range(K_TILES):
        nc.tensor.matmul(
            acc_ps[:, 0:132], oh[:, t, :], rhs[:, t, :],
            start=(t == 0), stop=(t == K_TILES - 1),
        )
    rn = outp.tile([128, HEAD], mybir.dt.float32, tag="rn")
    nc.vector.tensor_scalar_add(rn[:], acc_ps[:, 128:132], 1e-8)
    nc.vector.reciprocal(rn[:], rn[:])
    outb = outp.tile([128, LATDIM], mybir.dt.float16, tag="outb")
    nc.vector.tensor_tensor(
        outb[:].rearrange("p (h d) -> p h d", h=HEAD),
        acc_ps[:, 0:128].rearrange("p (h d) -> p h d", h=HEAD),
        rn[:].rearrange("p (h o) -> p h o", o=1)
        .to_broadcast([128, HEAD, HDIM]),
        op=mybir.AluOpType.mult,
    )
    nc.sync.dma_start(res[g * CAP_S:(g + 1) * CAP_S, :], outb[:])


# --------------------------------------------------------------------------
# entry point
# --------------------------------------------------------------------------

def _prepare(embeds, qTrans, kTrans, vTrans, filt, rows, cols):
    plans = [_plan_core(rows, cols, c * NLOC) for c in range(NCORES)]
    G = max(p["ngroups"] for p in plans)

    embh = embeds.astype(f16)
    filth = filt.astype(f16)

    qWh = np.ascontiguousarray(qTrans.astype(f16))
    kWh = np.ascontiguousarray(kTrans.astype(f16))
    vWh = np.ascontiguousarray(vTrans.astype(f16))
    hsel = np.zeros((LATDIM, HEAD), dtype=f16)
    for h in range(HEAD):
        hsel[h * HDIM:(h + 1) * HDIM, h] = 1.0
    s128 = np.arange(128, dtype=np.float32)

    in_maps = []
    for c in range(NCORES):
        p = plans[c]
        ng = p["ngroups"]

        scol = np.zeros(G * CAP_E, dtype=np.int64)
        scol[:ng * CAP_E] = p["cidx"].reshape(-1)
        sdst = np.zeros(G * CAP_E, dtype=np.int64)
        sdst[:ng * CAP_E] = p["didx"].reshape(-1)
        # [G, 128(d), 2, 768(e)]: transposed col/dest embeddings per slot
        colT = embh[scol].reshape(G, K_TILES * 128, 128)
        dstT = embh[sdst].reshape(G, K_TILES * 128, 128)
        ctabT = np.empty((G, 128, 2, CAP_E), dtype=f16)
        ctabT[:, :, 0, :] = colT.transpose(0, 2, 1)
        ctabT[:, :, 1, :] = dstT.transpose(0, 2, 1)

        # merged one-hot + filt [G, 128(e), 6, 132]: [:, :, :, 0:128] is
        # the one-hot scatter matrix, [:, :, :, 128:132] is filt[col_e]
        seg = np.full(G * CAP_E, PAD_SEG, dtype=np.float32)
        seg[:ng * CAP_E] = p["segrel"].reshape(-1)
        ohE = np.empty((G, 128, K_TILES, 132), dtype=bf16np)
        ohE[:, :, :, 0:128] = (
            seg.reshape(G, K_TILES, 128).transpose(0, 2, 1)[:, :, :, None]
            == s128[None, None, None, :]
        ).astype(bf16np)
        ohE[:, :, :, 128:132] = (
            filth[scol].reshape(G, K_TILES, 128, HEAD).transpose(0, 2, 1, 3)
        ).astype(bf16np)

        in_maps.append({
            "ctabT": ctabT,
            "ohE": ohE,
            "qW": qWh, "kW": kWh, "vW": vWh,
            "hsel": hsel,
        })
    return plans, G, in_maps


LAST_RESULT = None


def kernel(embeds, qTrans, kTrans, vTrans, filt, rows, cols, _trace=False):
    global LAST_RESULT
    from concourse.bass_utils import run_bass_kernel_spmd

    embeds = np.asarray(embeds, dtype=np.float32)
    qTrans = np.asarray(qTrans, dtype=np.float32)
    kTrans = np.asarray(kTrans, dtype=np.float32)
    vTrans = np.asarray(vTrans, dtype=np.float32)
    filt = np.asarray(filt, dtype=np.float32)
    rows = np.asarray(rows)
    cols = np.asarray(cols)

    plans, G, in_maps = _prepare(
        embeds, qTrans, kTrans, vTrans, filt, rows, cols
    )

    if G not in _CACHE:
        _CACHE[G] = _build_nc(G)
    nc = _CACHE[G]

    import os
    trace = _trace or bool(os.environ.get("GT_TRACE"))
    br = run_bass_kernel_spmd(nc, in_maps, core_ids=list(range(NCORES)),
                              trace=trace)
    LAST_RESULT = br

    out = np.zeros((N, LATDIM), dtype=np.float32)
    for c in range(NCORES):
        p = plans[c]
        dev = br.results[c]["res"]
        out[c * NLOC + p["remap_nodes"]] = dev[p["remap_rows"]].astype(np.float32)
    return out


# revision 43
# speedup vs baseline: 1.0853x; 1.0853x over previous
"""GTLayer (gnn_message_passing) Trainium2 kernel, v7.

Core-uniform single program, dest-sharded across 8 cores (12500 dest
nodes / ~75k edges per core; no collectives needed).

Per core: ~100 windows (<=128 dest segments x <=768 edges, 128-node window).

Host prep (layout only - gathers/permutes/transposes of input rows,
no arithmetic on values):
  - ctabT [G, 128, 2, 768] fp16: per-window TRANSPOSED per-edge-slot
    embeddings: [:, :, 0, e] = embeds[col_e], [:, :, 1, e] = embeds[dest_e].
  - ohE [G, 128, 6, 132] bf16: one-hot scatter matrix (cols 0:128) and
    filt[col_e] (cols 128:132), both in edge-partition layout.
All streamed to SBUF with plain HWDGE DMAs - no GPSIMD, no device-side
gathers (measured: SWDGE descriptor generation costs ~8ns/row on HW,
capping any device-gather design at ~650us for 75k edges/core).

Device per window (edge slots on the free axis, transposed layout):
  - qeT = qW^T @ destT, keT = kW^T @ colT (PSUM f32, 2 matmuls each)
  - ACT evacuates qeT -> SBUF fp16 (TT reads at most one PSUM input)
  - qkT = keT * qeT (DVE, fp16)
  - att[e, h] directly per 128-edge tile: matmul(lhsT=qkT chunk,
    rhs=hsel head-selector) -> acc[:, 132:156] - no [h, e] intermediate,
    no transpose-back matmuls
  - DVE clip(+-10) + filt add ([128, 24]); ACT exp -> rhs[:, :, 128:132]
  - ve = colT^T @ vW (PSUM, 6 matmuls); ACT evacuates ve -> SBUF bf16
    (rhs is off the critical path - the scatter delay absorbs the hop);
    DVE rhs = ve * expatt (bf16, all-SBUF)
  - scatter acc[s, 0:132] += oh^T @ [rhs | expatt] (6 matmuls, PSUM f32
    accumulation), delayed one window so the PE stays fed while DVE
    finishes the current window's rhs/oh
  - normalize by acc[:, 128:132] + 1e-8; fp16 out DMA (host casts back).
Host unpermutes window/segment rows to node rows at the end.

History: v2 (953us) per-edge indirect gathers + PE transposes; v4 (742us)
transposed dma_gather, GPSIMD-bound; v5 (515us) host-pretransposed
streams; v6 (313us) direct att matmuls; v7 (300us) merged oh/filt
stream + fp16 output; v12 (292us) split qts/qkT half-tiles + ACT ve
evacuation for an all-SBUF rhs multiply. Measured dead ends: Pool
engine rejects/corrupts elementwise ops; ACT hops on the latency-
critical qkT path regress; merging streams or PSUM tiles coarsens
dependencies and regresses; fp8 blocked by exp range / logit precision.
"""

import numpy as np
import ml_dtypes

N = 100000
E = 600000
LATDIM = 128
HEAD = 4
HDIM = 32
NCORES = 8
NLOC = N // NCORES              # 12500
CAP_S = 128
K_TILES = 6
CAP_E = K_TILES * 128           # 768
PAD_SEG = 999.0

f16 = np.float16
bf16np = ml_dtypes.bfloat16

_CACHE = {}


# --------------------------------------------------------------------------
# host-side planning
# --------------------------------------------------------------------------

def _plan_core(rows, cols, base):
    sel = (rows >= base) & (rows < base + NLOC)
    r = rows[sel].astype(np.int64) - base
    c = cols[sel].astype(np.int64)
    o = np.argsort(r, kind="stable")
    r, c = r[o], c[o]
    seg_nodes, seg_starts, seg_counts = np.unique(
        r, return_index=True, return_counts=True
    )
    nseg = len(seg_nodes)
    bounds = []
    lo, cur = 0, 0
    for i in range(nseg):
        cnt = int(seg_counts[i])
        if ((i - lo) + 1 > CAP_S or cur + cnt > CAP_E
                or seg_nodes[i] - seg_nodes[lo] >= CAP_S):
            bounds.append((lo, i))
            lo, cur = i, 0
        cur += cnt
    bounds.append((lo, nseg))
    ngroups = len(bounds)

    cidx = np.zeros((ngroups, CAP_E), dtype=np.int64)
    didx = np.zeros((ngroups, CAP_E), dtype=np.int64)   # dest node (global)
    segrel = np.full((ngroups, CAP_E), PAD_SEG, dtype=np.float32)
    remap_rows, remap_nodes = [], []
    for g, (slo, shi) in enumerate(bounds):
        e_lo = int(seg_starts[slo])
        e_hi = int(seg_starts[shi]) if shi < nseg else len(r)
        ne = e_hi - e_lo
        cidx[g, :ne] = c[e_lo:e_hi]
        didx[g, :ne] = r[e_lo:e_hi] + base
        rel = r[e_lo:e_hi] - seg_nodes[slo]
        segrel[g, :ne] = rel.astype(np.float32)
        remap_rows.append(g * CAP_S + (seg_nodes[slo:shi] - seg_nodes[slo]))
        remap_nodes.append(seg_nodes[slo:shi])
    return dict(
        ngroups=ngroups, cidx=cidx, didx=didx, segrel=segrel,
        remap_rows=np.concatenate(remap_rows),
        remap_nodes=np.concatenate(remap_nodes),
    )


# --------------------------------------------------------------------------
# device program
# --------------------------------------------------------------------------

def _build_nc(G):
    import concourse.bass as bass
    import concourse.mybir as mybir
    import concourse.tile as tile
    from concourse import bacc

    fp32 = mybir.dt.float32
    fp16 = mybir.dt.float16
    bf16 = mybir.dt.bfloat16

    nc = bacc.Bacc(None, target_bir_lowering=False, debug=True)

    ctabT = nc.dram_tensor("ctabT", [G, 128, 2, CAP_E], fp16,
                           kind="ExternalInput")
    ohE = nc.dram_tensor("ohE", [G, 128, K_TILES, 132], bf16,
                         kind="ExternalInput")
    qW = nc.dram_tensor("qW", [LATDIM, LATDIM], fp16, kind="ExternalInput")
    kW = nc.dram_tensor("kW", [LATDIM, LATDIM], fp16, kind="ExternalInput")
    vW = nc.dram_tensor("vW", [LATDIM, LATDIM], fp16, kind="ExternalInput")
    hsel = nc.dram_tensor("hsel", [LATDIM, HEAD], fp16, kind="ExternalInput")
    res = nc.dram_tensor("res", [G * CAP_S, LATDIM], fp16,
                         kind="ExternalOutput")

    with tile.TileContext(nc) as tc:
        with (
            tc.tile_pool(name="const", bufs=1) as constp,
            tc.tile_pool(name="gat", bufs=6) as gatp,
            tc.tile_pool(name="ohp", bufs=6) as ohp,
            tc.tile_pool(name="work", bufs=6) as workp,
            tc.tile_pool(name="ae", bufs=6) as aep,
            tc.tile_pool(name="wb", bufs=6) as wbp,
            tc.tile_pool(name="outp", bufs=6) as outp,
            tc.tile_pool(name="kv", bufs=3, space="PSUM") as kvp,
            tc.tile_pool(name="accps", bufs=2, space="PSUM") as accp,
        ):
            # ---- constants ----
            qW_sb = constp.tile([128, 128], fp16, tag="qW")
            nc.sync.dma_start(qW_sb[:], qW[:])
            kW_sb = constp.tile([128, 128], fp16, tag="kW")
            nc.sync.dma_start(kW_sb[:], kW[:])
            vW_sb = constp.tile([128, 128], fp16, tag="vW")
            nc.sync.dma_start(vW_sb[:], vW[:])
            hsel_sb = constp.tile([128, HEAD], fp16, tag="hsel")
            nc.sync.dma_start(hsel_sb[:], hsel[:])

            pend = None
            for g in range(G):
                ceT = gatp.tile([128, 2, CAP_E], fp16, tag="ceT")
                nc.sync.dma_start(ceT[:], ctabT[g])
                ohf = ohp.tile([128, K_TILES, 132], bf16, tag="ohf")
                nc.sync.dma_start(ohf[:], ohE[g])
                oh = ohf[:, :, 0:128]
                fe = ohf[:, :, 128:132]

                # qeT / keT (PSUM f32)
                qt_a = kvp.tile([128, 512], fp32, tag="KA")
                qt_b = kvp.tile([128, 256], fp32, tag="KB")
                nc.tensor.matmul(qt_a[:], qW_sb[:], ceT[:, 1, 0:512],
                                 start=True, stop=True)
                nc.tensor.matmul(qt_b[:], qW_sb[:], ceT[:, 1, 512:768],
                                 start=True, stop=True)
                kt_a = kvp.tile([128, 512], fp32, tag="KA")
                kt_b = kvp.tile([128, 256], fp32, tag="KB")
                nc.tensor.matmul(kt_a[:], kW_sb[:], ceT[:, 0, 0:512],
                                 start=True, stop=True)
                nc.tensor.matmul(kt_b[:], kW_sb[:], ceT[:, 0, 512:768],
                                 start=True, stop=True)
                # evac qt to SBUF (ACT) — TT may read only one PSUM input;
                # split tiles so att tiles 0-3 depend only on the a-half
                qts_a = workp.tile([128, 512], fp16, tag="qtsa")
                qts_b = workp.tile([128, 256], fp16, tag="qtsb")
                nc.scalar.copy(qts_a[:], qt_a[:])
                nc.scalar.copy(qts_b[:], qt_b[:])
                # qkT (DVE, fp16)
                qkT_a = workp.tile([128, 512], fp16, tag="qkta")
                qkT_b = workp.tile([128, 256], fp16, tag="qktb")
                nc.vector.tensor_tensor(qkT_a[:], kt_a[:], qts_a[:],
                                        op=mybir.AluOpType.mult)
                nc.vector.tensor_tensor(qkT_b[:], kt_b[:], qts_b[:],
                                        op=mybir.AluOpType.mult)
                # att[e, h] directly: per tile, lhsT=qkT chunk, rhs=hsel
                acc_ps = accp.tile([128, 156], fp32, tag="acc")
                for t in range(K_TILES):
                    if t < 4:
                        lhs = qkT_a[:, t * 128:(t + 1) * 128]
                    else:
                        lhs = qkT_b[:, (t - 4) * 128:(t - 3) * 128]
                    nc.tensor.matmul(
                        acc_ps[:, 132 + t * 4:132 + (t + 1) * 4],
                        lhs,
                        hsel_sb[:],
                        start=True, stop=True,
                    )
                # clip + filt (DVE, [128, 24])
                ae = aep.tile([128, K_TILES, HEAD], fp16, tag="ae")
                nc.vector.tensor_scalar(
                    ae[:],
                    acc_ps[:, 132:156].rearrange("p (t h) -> p t h", h=HEAD),
                    10.0, -10.0,
                    op0=mybir.AluOpType.min, op1=mybir.AluOpType.max,
                )
                nc.vector.tensor_tensor(ae[:], ae[:], fe,
                                        op=mybir.AluOpType.add)
                rhs = wbp.tile([128, K_TILES, LATDIM + HEAD], bf16, tag="rhs")
                nc.scalar.activation(
                    rhs[:, :, 128:132], ae[:],
                    mybir.ActivationFunctionType.Exp,
                )
                # ve (PSUM)
                ve_a = kvp.tile([128, 512], fp32, tag="KA")
                ve_b = kvp.tile([128, 256], fp32, tag="KB")
                for t in range(K_TILES):
                    if t < 4:
                        vout = ve_a[:, t * 128:(t + 1) * 128]
                    else:
                        vout = ve_b[:, (t - 4) * 128:(t - 3) * 128]
                    nc.tensor.matmul(
                        vout, ceT[:, 0, t * 128:(t + 1) * 128], vW_sb[:],
                        start=True, stop=True,
                    )
                # evac ve to SBUF bf16 (ACT) so the rhs mult gets the DVE
                # all-SBUF fast mode; the one-window scatter delay absorbs
                # the extra latency hop
                ves = wbp.tile([128, CAP_E], bf16, tag="ves")
                nc.scalar.copy(ves[:, 0:512], ve_a[:])
                nc.scalar.copy(ves[:, 512:768], ve_b[:])
                # pending scatter from previous window (PE fill)
                if pend is not None:
                    _emit_scatter(nc, mybir, outp, res, *pend)
                # rhs = ve * expatt (DVE, all-SBUF)
                nc.vector.tensor_tensor(
                    rhs[:, :, 0:128].rearrange("p t (h d) -> p t h d", h=HEAD),
                    ves[:].rearrange("p (t h d) -> p t h d", t=K_TILES, h=HEAD),
                    rhs[:, :, 128:132].rearrange("p t (h o) -> p t h o", o=1)
                    .to_broadcast([128, K_TILES, HEAD, HDIM]),
                    op=mybir.AluOpType.mult,
                )
                pend = (g, acc_ps, oh, rhs)
            _emit_scatter(nc, mybir, outp, res, *pend)

    nc.compile()
    return nc


def _emit_scatter(nc, mybir, outp, res, g, acc_ps, oh, rhs):
    for t in range(K_TILES):
        nc.tensor.matmul(
            acc_ps[:, 0:132], oh[:, t, :], rhs[:, t, :],
            start=(t == 0), stop=(t == K_TILES - 1),
        )
    rn = outp.tile([128, HEAD], mybir.dt.float32, tag="rn")
    nc.vector.tensor_scalar_add(rn[:], acc_ps[:, 128:132], 1e-8)
    nc.vector.reciprocal(rn[:], rn[:])
    outb = outp.tile([128, LATDIM], mybir.dt.float16, tag="outb")
    nc.vector.tensor_tensor(
        outb[:].rearrange("p (h d) -> p h d", h=HEAD),
        acc_ps[:, 0:128].rearrange("p (h d) -> p h d", h=HEAD),
        rn[:].rearrange("p (h o) -> p h o", o=1)
        .to_broadcast([128, HEAD, HDIM]),
        op=mybir.AluOpType.mult,
    )
    nc.sync.dma_start(res[g * CAP_S:(g + 1) * CAP_S, :], outb[:])


# --------------------------------------------------------------------------
# entry point
# --------------------------------------------------------------------------

def _prepare(embeds, qTrans, kTrans, vTrans, filt, rows, cols):
    plans = [_plan_core(rows, cols, c * NLOC) for c in range(NCORES)]
    G = max(p["ngroups"] for p in plans)

    embh = embeds.astype(f16)
    filth = filt.astype(f16)

    qWh = np.ascontiguousarray(qTrans.astype(f16))
    kWh = np.ascontiguousarray(kTrans.astype(f16))
    vWh = np.ascontiguousarray(vTrans.astype(f16))
    hsel = np.zeros((LATDIM, HEAD), dtype=f16)
    for h in range(HEAD):
        hsel[h * HDIM:(h + 1) * HDIM, h] = 1.0
    s128 = np.arange(128, dtype=np.float32)

    in_maps = []
    for c in range(NCORES):
        p = plans[c]
        ng = p["ngroups"]

        scol = np.zeros(G * CAP_E, dtype=np.int64)
        scol[:ng * CAP_E] = p["cidx"].reshape(-1)
        sdst = np.zeros(G * CAP_E, dtype=np.int64)
        sdst[:ng * CAP_E] = p["didx"].reshape(-1)
        # [G, 128(d), 2, 768(e)]: transposed col/dest embeddings per slot
        colT = embh[scol].reshape(G, K_TILES * 128, 128)
        dstT = embh[sdst].reshape(G, K_TILES * 128, 128)
        ctabT = np.empty((G, 128, 2, CAP_E), dtype=f16)
        ctabT[:, :, 0, :] = colT.transpose(0, 2, 1)
        ctabT[:, :, 1, :] = dstT.transpose(0, 2, 1)

        # merged one-hot + filt [G, 128(e), 6, 132]: [:, :, :, 0:128] is
        # the one-hot scatter matrix, [:, :, :, 128:132] is filt[col_e]
        seg = np.full(G * CAP_E, PAD_SEG, dtype=np.float32)
        seg[:ng * CAP_E] = p["segrel"].reshape(-1)
        ohE = np.empty((G, 128, K_TILES, 132), dtype=bf16np)
        ohE[:, :, :, 0:128] = (
            seg.reshape(G, K_TILES, 128).transpose(0, 2, 1)[:, :, :, None]
            == s128[None, None, None, :]
        ).astype(bf16np)
        ohE[:, :, :, 128:132] = (
            filth[scol].reshape(G, K_TILES, 128, HEAD).transpose(0, 2, 1, 3)
        ).astype(bf16np)

        in_maps.append({
            "ctabT": ctabT,
            "ohE": ohE,
            "qW": qWh, "kW": kWh, "vW": vWh,
            "hsel": hsel,
        })
    return plans, G, in_maps


LAST_RESULT = None


def kernel(embeds, qTrans, kTrans, vTrans, filt, rows, cols, _trace=False):
    global LAST_RESULT
    from concourse.bass_utils import run_bass_kernel_spmd

    embeds = np.asarray(embeds, dtype=np.float32)
    qTrans = np.asarray(qTrans, dtype=np.float32)
    kTrans = np.asarray(kTrans, dtype=np.float32)
    vTrans = np.asarray(vTrans, dtype=np.float32)
    filt = np.asarray(filt, dtype=np.float32)
    rows = np.asarray(rows)
    cols = np.asarray(cols)

    plans, G, in_maps = _prepare(
        embeds, qTrans, kTrans, vTrans, filt, rows, cols
    )

    if G not in _CACHE:
        _CACHE[G] = _build_nc(G)
    nc = _CACHE[G]

    import os
    trace = _trace or bool(os.environ.get("GT_TRACE"))
    br = run_bass_kernel_spmd(nc, in_maps, core_ids=list(range(NCORES)),
                              trace=trace)
    LAST_RESULT = br

    out = np.zeros((N, LATDIM), dtype=np.float32)
    for c in range(NCORES):
        p = plans[c]
        dev = br.results[c]["res"]
        out[c * NLOC + p["remap_nodes"]] = dev[p["remap_rows"]].astype(np.float32)
    return out
